# revision 50
# baseline (speedup 1.0000x reference)
"""Multi-head self-attention (B=1, S=4096, D=512, H=8) on 8 trn2 NeuronCores.

Sharding: one head per core (head/tensor parallel). Each core computes its
head's Q/K/V projections from the full (transposed) query, runs attention
without materializing the full score matrix (streaming over key chunks,
softmax denominator via a ones-column augmented V^T), applies its slice of
out_proj fused with softmax normalization, and writes an unnormalized partial
[S, D] output. Host sums the 8 partials and adds out_proj bias.

All matmul operands are bf16 (f32 PSUM accumulate): the PE streams bf16
moving operands at 1 col/cycle vs ~2 cycles for f32r, and input/output HBM
traffic halves. The softmax scale is folded into wq on the host. Query DMA
is chunked per 512-column group so many DMA engines run in parallel and
projections start before the full load lands. out_proj + normalization +
output DMA for group g are interleaved into group g+1's attention batches.
"""

import sys

sys.path.insert(0, "/opt/trn_rl_repo")

import numpy as np
import ml_dtypes

BF = ml_dtypes.bfloat16

EMBED = 512
HEADS = 8
HD = 64          # head dim
S = 4096         # sequence length
P = 128          # partitions
NSK = S // P     # 32 key chunks of 128
QG = 512         # query group width (matmul free dim)
NQG = S // QG    # 8 query groups
NDC = EMBED // P # 4 contraction chunks for projections
SCALE = HD ** -0.5
EXP_BATCH = 3    # key chunks per exp batch (PSUM banks per slot)

_compiled = {}


def _build(n_cores=8):
    import concourse.bacc as bacc
    import concourse.mybir as mybir
    import concourse.tile as tile

    f32 = mybir.dt.float32
    bf16 = mybir.dt.bfloat16

    nc = bacc.Bacc("TRN2", target_bir_lowering=False, debug=False,
                   num_devices=n_cores)

    qt = nc.dram_tensor("qt", [EMBED, S], bf16, kind="ExternalInput")
    # [wq*scale | 0pad | wk | 0pad | wv] packed: one DMA per 128-row chunk
    wqkv = nc.dram_tensor("wqkv", [EMBED, 2 * P + HD], bf16,
                          kind="ExternalInput")
    wo = nc.dram_tensor("wo", [P, EMBED], bf16, kind="ExternalInput")
    bqk = nc.dram_tensor("bqk", [P, 2], f32, kind="ExternalInput")
    bv = nc.dram_tensor("bv", [P, HD], f32, kind="ExternalInput")
    out_p = nc.dram_tensor("out_p", [S, EMBED], bf16, kind="ExternalOutput")

    with tile.TileContext(nc) as tc:
        _emit(tc, nc, mybir, qt, wqkv, wo, bqk, bv, out_p)

    nc.compile()
    return nc


def _emit(tc, nc, mybir, qt, wqkv, wo, bqk, bv, out_p):
    from contextlib import ExitStack

    f32 = mybir.dt.float32
    bf16 = mybir.dt.bfloat16
    Exp = mybir.ActivationFunctionType.Exp
    Copy = mybir.ActivationFunctionType.Copy

    with ExitStack() as ctx:
        singles = ctx.enter_context(tc.tile_pool(name="singles", bufs=1))

        # --- warm up the ACT exp table while DMAs run ---
        warm = singles.tile([1, 1], f32)
        nc.vector.memset(warm, 0.0)
        warm2 = singles.tile([1, 1], f32)
        nc.scalar.activation(warm2, warm, Exp)
        one_sb = singles.tile([1, 1], f32)
        nc.vector.memset(one_sb, 1.0)

        # --- query load. Dispatch cost (~700ns per dma_start, descriptors
        # then spread across all 16 DMA engines) dominates landing time, so:
        # early groups get per-group DMAs (prompt start), late groups share
        # wide 3-group DMAs (fewer dispatches), weights ride packed. ---
        qt_sb = [singles.tile([P, S], bf16, tag=f"qt{c}", name=f"qt_sb{c}")
                 for c in range(NDC)]
        # [wq|0, wk|0, wv] packed; zero-padding keeps every matmul on a
        # (128,128) PE tile, avoiding the ~130ns reconfig on shape switches
        wqkv_sb = singles.tile([P, NDC, 2 * P + HD], bf16)
        wq_s = [wqkv_sb[:, c, 0:P] for c in range(NDC)]
        wk_s = [wqkv_sb[:, c, P:2 * P] for c in range(NDC)]
        wv_s = [wqkv_sb[:, c, 2 * P:2 * P + HD] for c in range(NDC)]
        dq = [nc.sync, nc.gpsimd, nc.scalar]
        di = 0

        def load_qt(g0, ng):
            nonlocal di
            gsl = slice(g0 * QG, (g0 + ng) * QG)
            for c in range(NDC):
                dq[di % 3].dma_start(out=qt_sb[c][:, gsl],
                                     in_=qt[c * P:(c + 1) * P, gsl])
                di += 1

        # tiny transfers first: they absorb each queue's DGE spin-up so the
        # bandwidth-bound qt stream starts at full rate
        bqk_sb = singles.tile([P, 2], f32)
        nc.scalar.dma_start(out=bqk_sb, in_=bqk[:, :])
        bv_sb = singles.tile([P, HD], f32)
        nc.sync.dma_start(out=bv_sb, in_=bv[:, :])
        bq_sb = bqk_sb[:, 0:1]
        bk_sb = bqk_sb[:, 1:2]
        load_qt(0, 1)
        for c in range(NDC):
            dq[(c + 1) % 3].dma_start(out=wqkv_sb[:, c, :],
                                      in_=wqkv[c * P:(c + 1) * P, :])
        load_qt(1, 1)
        wo_sb = singles.tile([P, EMBED], bf16)
        nc.gpsimd.dma_start(out=wo_sb, in_=wo[:, :])
        load_qt(2, 3)
        load_qt(5, 3)

        # persistent activations; q/k/ot rows 64-127 are zero so the
        # attention matmuls all use full-K (128,128) PE tiles. The zeros
        # come for free: padded projection stationaries produce zero PSUM
        # rows, and full-height evictions carry them along. vt cols 65-127
        # are zeroed once (gpsimd) so out_acc rows 65-127 are zero; the
        # denominator lands in ot row 64 and is annihilated by wo's zero
        # rows in the out-projection.
        q_sb = singles.tile([P, S], bf16)       # Q^T (pre-scaled): [hd, s]
        k_sb = singles.tile([P, S], bf16)       # K^T: [hd, s]
        vt_sb = singles.tile([P, NSK, P], bf16)  # V^T chunks + ones + 0pad
        ot_sb = singles.tile([P, S], bf16)      # unnormalized attn out^T
        den_row = singles.tile([1, S], f32)     # denominator, row layout
        den_all = singles.tile([P, NSK], bf16)  # denominator, [sq%128, chunk]
        recip_all = singles.tile([P, NSK], f32) # 1/denominator

        nc.gpsimd.memset(vt_sb[:, :, HD:HD + 1], 1.0)
        nc.gpsimd.memset(vt_sb[:, :, HD + 1:P], 0.0)

        # --- stage B: projections ---
        with ExitStack() as bctx:
            pqk = bctx.enter_context(
                tc.tile_pool(name="pqk", bufs=2, space="PSUM"))
            pvp = bctx.enter_context(
                tc.tile_pool(name="pvp", bufs=2, space="PSUM"))

            # per landed query group: K, Q, then V of that group's chunks, so
            # the PE always has work while later groups' DMAs stream in
            for g in range(NQG):
                sl = slice(g * QG, (g + 1) * QG)
                acc_k = pqk.tile([P, QG], f32, tag="pj")
                for c in range(NDC):
                    nc.tensor.matmul(acc_k, wk_s[c], qt_sb[c][:, sl],
                                     start=(c == 0), stop=(c == NDC - 1))
                nc.vector.tensor_scalar_add(k_sb[:, sl], acc_k, bk_sb)
                acc_q = pqk.tile([P, QG], f32, tag="pj")
                for c in range(NDC):
                    nc.tensor.matmul(acc_q, wq_s[c], qt_sb[c][:, sl],
                                     start=(c == 0), stop=(c == NDC - 1))
                nc.vector.tensor_scalar_add(q_sb[:, sl], acc_q, bq_sb)
                for i in range(QG // P):
                    s = g * (QG // P) + i
                    ssl = slice(s * P, (s + 1) * P)
                    acc_v = pvp.tile([P, HD], f32, tag="pv")
                    for c in range(NDC):
                        nc.tensor.matmul(acc_v, qt_sb[c][:, ssl], wv_s[c],
                                         start=(c == 0), stop=(c == NDC - 1))
                    nc.vector.tensor_add(vt_sb[:, s, 0:HD], acc_v, bv_sb)

            # low-priority filler matmuls: the scheduler drops them into the
            # DMA-starvation gaps of stage B, keeping the PE's DVFS ramp hot
            # (idle resets it to ~1.2GHz; ramped it runs 2.4GHz). Their PSUM
            # tiles come from a stage-B pool, so the attention pools' bank
            # reuse bounds them: they cannot spill past the start of
            # attention.
            dum_pool = bctx.enter_context(
                tc.tile_pool(name="dum_pool", bufs=2, space="PSUM"))
            for d in range(0):
                dps = dum_pool.tile([P, QG], f32, tag="dum")
                nc.tensor.matmul(dps, wq_s[0], qt_sb[0][:, 0:QG],
                                 start=True, stop=True)

        # --- attention + fused out_proj epilogue ---
        with ExitStack() as cctx:
            s_pool = cctx.enter_context(
                tc.tile_pool(name="s_pool", bufs=2, space="PSUM"))
            acc_pool = cctx.enter_context(
                tc.tile_pool(name="acc_pool", bufs=1, space="PSUM"))
            op_pool = cctx.enter_context(
                tc.tile_pool(name="op_pool", bufs=1, space="PSUM"))
            p_pool = cctx.enter_context(tc.tile_pool(name="p_pool", bufs=6))
            o_pool = cctx.enter_context(tc.tile_pool(name="o_pool", bufs=3))

            oq = [nc.sync, nc.gpsimd]

            def epilogue(g, out_acc, u):
                gsl = slice(g * QG, (g + 1) * QG)
                last = g == NQG - 1
                if u == 0:
                    # single eviction: the bf16 numerator cast carries the
                    # denominator (row 64) and zero rows 65-127 along, so
                    # the next group's accumulator frees after ONE DVE op
                    nc.vector.tensor_copy(ot_sb[:, gsl], out_acc)
                    if last:
                        # tail: a DMA transpose costs ~3.5us in completion
                        # semaphores; transpose via the PE instead. f32 den
                        # here (one extra copy, off the critical boundary)
                        nc.vector.tensor_copy(den_row[:, gsl],
                                              out_acc[HD:HD + 1, :])
                        dt_ps = op_pool.tile([P, QG // P], f32, tag="op")
                        for i in range(QG // P):
                            j = g * (QG // P) + i
                            nc.tensor.matmul(
                                dt_ps[:, i:i + 1],
                                den_row[0:1, j * P:(j + 1) * P],
                                one_sb, is_transpose=True,
                                start=True, stop=True)
                        nc.vector.reciprocal(
                            recip_all[:, g * (QG // P):(g + 1) * (QG // P)],
                            dt_ps)
                    else:
                        for i in range(QG // P):
                            j = g * (QG // P) + i
                            oq[i % 2].dma_start(
                                out=den_all[:, j:j + 1],
                                in_=ot_sb[HD:HD + 1, j * P:(j + 1) * P])
                        nc.vector.reciprocal(
                            recip_all[:, g * (QG // P):(g + 1) * (QG // P)],
                            den_all[:, g * (QG // P):(g + 1) * (QG // P)])
                else:
                    t = g * (QG // P) + (u - 1)
                    tsl = slice(t * P, (t + 1) * P)
                    if last:
                        # attention is done: the score-pool banks are free,
                        # use them so the four tail matmuls double-buffer
                        o_ps = s_pool.tile([P, EMBED], f32, tag="sps")
                    else:
                        o_ps = op_pool.tile([P, EMBED], f32, tag="op")
                    nc.tensor.matmul(o_ps, ot_sb[:, tsl], wo_sb,
                                     start=True, stop=True)
                    o_sb = o_pool.tile([P, EMBED], bf16, tag="o")
                    if last and u % 2 == 0:
                        # ACT is idle once the final exp is done: normalize
                        # + evict via activation(Copy, scale=1/den)
                        nc.scalar.activation(o_sb, o_ps, Copy,
                                             scale=recip_all[:, t:t + 1])
                    else:
                        nc.vector.tensor_scalar_mul(o_sb, o_ps,
                                                    recip_all[:, t:t + 1])
                    oq[t % 2].dma_start(out=out_p[tsl, :], in_=o_sb)

            def batch_list(g):
                if g == NQG - 1:
                    # final group: taper the last batches so the S->exp->AV
                    # drain at kernel end is shorter
                    sizes = [EXP_BATCH] * ((NSK - 2) // EXP_BATCH) + [1, 1]
                else:
                    sizes = [EXP_BATCH] * (NSK // EXP_BATCH)
                    if NSK % EXP_BATCH:
                        sizes.append(NSK % EXP_BATCH)
                out, start = [], 0
                for s in sizes:
                    out.append(list(range(start, start + s)))
                    start += s
                return out

            # flat batch stream: pairs may straddle group boundaries so
            # neither engine sees a per-group pipeline bubble. Pairing
            # ([S,S][exp,exp][AV,AV]) halves the PE's stationary-switch
            # penalties (~130ns per run start).
            all_b = [(g, chunks)
                     for g in range(NQG) for chunks in batch_list(g)]
            out_accs = {}   # g -> psum tile
            pend = []       # deferred out-proj units: (g, out_acc, unit)
            prev = None

            def start_group(g):
                nonlocal prev
                if prev is not None:
                    epilogue(prev[0], prev[1], 0)
                    pend.extend((prev[0], prev[1], u)
                                for u in range(1, QG // P + 1))
                out_accs[g] = acc_pool.tile([P, QG], f32, tag="acc",
                                            name=f"out_acc{g}")
                prev = (g, out_accs[g])

            for idx in range(0, len(all_b), 2):
                pair_src = all_b[idx:idx + 2]
                pair = []
                for g, chunks in pair_src:
                    gsl = slice(g * QG, (g + 1) * QG)
                    s_ps = s_pool.tile([P, EXP_BATCH * QG], f32, tag="sps")
                    for i, s in enumerate(chunks):
                        nc.tensor.matmul(
                            s_ps[:, i * QG:(i + 1) * QG],
                            k_sb[:, s * P:(s + 1) * P], q_sb[:, gsl],
                            start=True, stop=True)
                    pair.append((g, chunks, s_ps))
                ppair = []
                for g, chunks, s_ps in pair:
                    nb = len(chunks)
                    p_sb = p_pool.tile([P, EXP_BATCH * QG], bf16, tag="p")
                    nc.scalar.activation(p_sb[:, :nb * QG],
                                         s_ps[:, :nb * QG], Exp)
                    ppair.append((g, chunks, p_sb))
                for g, chunks, p_sb in ppair:
                    if chunks[0] == 0:
                        start_group(g)
                    for i, s in enumerate(chunks):
                        nc.tensor.matmul(
                            out_accs[g], vt_sb[:, s, :],
                            p_sb[:, i * QG:(i + 1) * QG],
                            start=(s == 0), stop=(s == NSK - 1))
                if pend:
                    epilogue(*pend.pop(0))

            # final group's epilogue
            epilogue(prev[0], prev[1], 0)
            for u in range(1, QG // P + 1):
                epilogue(prev[0], prev[1], u)
            while pend:
                epilogue(*pend.pop(0))


def _in_maps(query, in_proj_weight, in_proj_bias, out_proj_weight):
    q2d = np.asarray(query, dtype=np.float32).reshape(S, EMBED)
    qt = np.ascontiguousarray(q2d.T).astype(BF)
    w = np.asarray(in_proj_weight, dtype=np.float32)
    b = np.asarray(in_proj_bias, dtype=np.float32)
    wout = np.asarray(out_proj_weight, dtype=np.float32)
    maps = []
    for h in range(HEADS):
        hs = slice(h * HD, (h + 1) * HD)
        ks = slice(EMBED + h * HD, EMBED + (h + 1) * HD)
        vs = slice(2 * EMBED + h * HD, 2 * EMBED + (h + 1) * HD)
        # [wq*scale | 0pad64 | wk | 0pad64 | wv]; pads keep the projection
        # stationaries (128,128) PE tiles
        wqkv = np.zeros((EMBED, 2 * P + HD), dtype=np.float32)
        wqkv[:, 0:HD] = w[hs, :].T * SCALE
        wqkv[:, P:P + HD] = w[ks, :].T
        wqkv[:, 2 * P:2 * P + HD] = w[vs, :].T
        wo = np.zeros((P, EMBED), dtype=np.float32)
        wo[0:HD, :] = wout[:, hs].T
        bqk = np.zeros((P, 2), dtype=np.float32)
        bqk[0:HD, 0] = b[hs] * SCALE
        bqk[0:HD, 1] = b[ks]
        maps.append({
            "qt": qt,
            "wqkv": wqkv.astype(BF),
            "wo": wo.astype(BF),
            "bqk": np.ascontiguousarray(bqk).astype(np.float32),
            "bv": np.ascontiguousarray(
                np.broadcast_to(b[vs], (P, HD))).astype(np.float32),
        })
    return maps


def get_nc():
    if "nc" not in _compiled:
        _compiled["nc"] = _build()
    return _compiled["nc"]


def kernel(query, in_proj_weight, in_proj_bias, out_proj_weight, out_proj_bias):
    from concourse.bass_utils import run_bass_kernel_spmd

    nc = get_nc()
    maps = _in_maps(query, in_proj_weight, in_proj_bias, out_proj_weight)
    res = run_bass_kernel_spmd(nc, maps, core_ids=list(range(HEADS)))
    acc = np.zeros((S, EMBED), dtype=np.float32)
    for h in range(HEADS):
        acc += np.asarray(res.results[h]["out_p"], dtype=np.float32)
    acc += np.asarray(out_proj_bias, dtype=np.float32)[None, :]
    return acc.reshape(np.asarray(query).shape).astype(np.float32)
